# revision 5
# baseline (speedup 1.0000x reference)
"""Candidate kernel.py: grouped-gather version (self-contained)."""

import math

import numpy as np

N_NODES = 10000
HV_DIM = 10000
ALPHA = 0.85
PR_ITERS = 10

NROWS = N_NODES + 1  # row N_NODES is all-zeros (padding target)
EW = 1280  # per-core D-slice width (8 * 1280 = 10240 >= 10000)
BLK = 256  # groups per gather block
G = BLK // 128
N_CORES = 8
KINDS = (4, 2, 1)
MAXM = max(KINDS)


def _host_indices(edge_index: np.ndarray) -> tuple[np.ndarray, np.ndarray]:
    """Pagerank -> rank permutation -> deduped undirected edge endpoint rows.
    Bit-exact replica of the reference ops on the CPU jax backend."""
    import jax
    import jax.numpy as jnp
    from jax import lax

    N = N_NODES
    cpu = jax.devices("cpu")[0]

    def _impl(edge_index):
        row, col = edge_index[0], edge_index[1]
        dtype = jnp.float32
        counts = jax.ops.segment_sum(
            jnp.ones_like(col, dtype=dtype), col, num_segments=N
        )
        vals = ALPHA / counts[col]
        p = jnp.asarray((1.0 - ALPHA) / N, dtype=dtype)
        v0 = jnp.full((N,), 1.0 / N, dtype=dtype)

        def step(v, _):
            v = jax.ops.segment_sum(vals * v[col], row, num_segments=N) + p
            return v, None

        v, _ = lax.scan(step, v0, None, length=PR_ITERS)

        perm = jnp.argsort(v)
        inv = (
            jnp.zeros((N,), dtype=jnp.int32)
            .at[perm]
            .set(jnp.arange(N, dtype=jnp.int32))
        )

        lo = jnp.minimum(row, col)
        hi = jnp.maximum(row, col)
        ekey = lo * jnp.int32(N) + hi
        order = jnp.argsort(ekey)
        skey = ekey[order]
        first = jnp.concatenate([jnp.ones((1,), dtype=bool), skey[1:] != skey[:-1]])
        slo = lo[order]
        shi = hi[order]
        return inv[slo], inv[shi], first

    with jax.default_device(cpu):
        ei = jax.device_put(np.asarray(edge_index), cpu)
        try:
            fn = jax.jit(_impl, backend="cpu")
        except TypeError:
            fn = jax.jit(_impl)
        ia, ib, first = fn(ei)
        ia, ib, first = np.asarray(ia), np.asarray(ib), np.asarray(first)
    return ia[first], ib[first]


def _plan_groups(a: np.ndarray, b: np.ndarray):
    """Orient each edge toward its higher-degree endpoint, chunk each node's
    partner list into quads/pairs/singles (then regroup leftover singles by
    the other endpoint), and emit the flat gather schedule.

    Returns (schedule, idx_cols): schedule = [(m, [col offsets of U, W1..Wm])],
    idx_cols = packed [128, S] int16 gather-index tensor."""
    deg = np.bincount(np.concatenate([a, b]), minlength=NROWS)
    swap = deg[b] > deg[a]
    u = np.where(swap, b, a)
    w = np.where(swap, a, b)
    order = np.argsort(u, kind="stable")
    u, w = u[order], w[order]

    def chunk_runs(keys, vals):
        out = {m: [] for m in KINDS}
        i = 0
        n = len(keys)
        while i < n:
            j = i
            while j < n and keys[j] == keys[i]:
                j += 1
            run = vals[i:j]
            k = 0
            for m in KINDS:
                while len(run) - k >= m:
                    out[m].append((keys[i], run[k : k + m]))
                    k += m
            i = j
        return out

    groups = chunk_runs(u, w)
    singles = groups[1]
    if singles:  # products commute: regroup singles by the partner endpoint
        su = np.array([g[0] for g in singles])
        sw = np.array([g[1][0] for g in singles])
        o2 = np.argsort(sw, kind="stable")
        regrouped = chunk_runs(sw[o2], su[o2])
        groups[1] = regrouped.pop(1)
        for m in KINDS[:-1]:
            groups[m].extend(regrouped[m])

    idx_stream = []
    schedule = []
    col = 0

    def emit_gather(vals):
        nonlocal col
        arr = np.full(BLK, N_NODES, dtype=np.int16)
        arr[: len(vals)] = vals
        idx_stream.append(arr)
        off = col
        col += BLK // 16
        return off

    for m in KINDS:
        g = groups[m]
        for bi in range(math.ceil(len(g) / BLK)):
            chunk = g[bi * BLK : (bi + 1) * BLK]
            offs = [emit_gather(np.array([c[0] for c in chunk], dtype=np.int16))]
            for j in range(m):
                offs.append(
                    emit_gather(np.array([c[1][j] for c in chunk], dtype=np.int16))
                )
            schedule.append((m, offs))

    idx_cols = np.zeros((128, col), dtype=np.int16)
    for k, arr in enumerate(idx_stream):
        wrapped = arr.reshape(-1, 16).T
        idx_cols[:, k * 16 : (k + 1) * 16] = np.tile(wrapped, (8, 1))
    return schedule, idx_cols


_NC_CACHE: dict = {}


def _build_nc(schedule, scols: int):
    """Per-core bass program processing the gather schedule.
    Per block (kind m): gather U + W1..Wm (BLK idxs each), then DVE computes
    acc += U * (W1 + ... + Wm)."""
    key = (scols, tuple((m, tuple(o)) for m, o in schedule))
    if key in _NC_CACHE:
        return _NC_CACHE[key]

    from contextlib import ExitStack

    import concourse.bacc as bacc
    import concourse.bass as bass
    import concourse.mybir as mybir
    from concourse import library_config
    from concourse._compat import get_trn_type

    DT = mybir.dt.float32
    nc = bacc.Bacc(get_trn_type() or "TRN2")
    table = nc.dram_tensor("table", [NROWS, EW], DT, kind="ExternalInput")
    idx = nc.dram_tensor("idx", [128, scols], mybir.dt.int16, kind="ExternalInput")
    out = nc.dram_tensor("out", [128, G, EW], DT, kind="ExternalOutput")

    ops_of = lambda m: 2 if m == 1 else m + 1  # noqa: E731
    done_ops = []
    tot = 0
    for m, _ in schedule:
        tot += ops_of(m)
        done_ops.append(tot)
    total_ops = tot
    NSTREAM = MAXM + 1

    with (
        ExitStack() as stack,
        nc.sbuf_tensor("idx_sb", [128, scols], mybir.dt.int16) as idx_sb,
        nc.sbuf_tensor("gbuf", [128, NSTREAM, 2, G, EW], DT) as gbuf,
        nc.sbuf_tensor("tsum", [128, G, EW], DT) as tsum,
        nc.sbuf_tensor("acc", [128, G, EW], DT) as acc,
        nc.semaphore("io_sem") as io_sem,
        nc.semaphore("o_sem") as o_sem,
        nc.semaphore("vop_sem") as vop_sem,
        nc.Block() as block,
    ):
        gsem = [
            [stack.enter_context(nc.semaphore(f"g{j}_{s}")) for s in range(2)]  # noqa: ANT232
            for j in range(NSTREAM)
        ]

        # deterministic (stream, slot) rotation shared by both engine streams
        def walk():
            use = [[0, 0] for _ in range(NSTREAM)]
            hist = [[[], []] for _ in range(NSTREAM)]
            for bi, (m, offs) in enumerate(schedule):
                entries = []
                for jj in range(m + 1):
                    cnt = use[jj][0] + use[jj][1]
                    s = cnt % 2
                    use[jj][s] += 1
                    prior = hist[jj][s][:]
                    hist[jj][s].append(bi)
                    entries.append((jj, s, use[jj][s], prior, offs[jj]))
                yield bi, m, entries

        @block.gpsimd
        def _(gp: bass.BassGpSimd):
            gp.load_library(library_config.mlp)
            gp.dma_start(idx_sb[:, :], idx[:, :]).then_inc(io_sem, 16)
            gp.wait_ge(io_sem, 16)
            for bi, m, entries in walk():
                for jj, s, k, prior, off in entries:
                    if prior:
                        # slot reuse: DVE must have consumed block prior[-1]
                        gp.wait_ge(vop_sem, done_ops[prior[-1]])
                    gp.dma_gather(
                        gbuf[:, jj, s],
                        table[:, :],
                        idx_sb[:, bass.ds(off, BLK // 16)],
                        BLK,
                        BLK,
                        EW,
                    ).then_inc(gsem[jj][s], 16)

        @block.vector
        def _(v: bass.BassEngine):
            nops = 0
            for bi, m, entries in walk():
                for jj, s, k, prior, off in entries:
                    v.wait_ge(gsem[jj][s], 16 * k)

                def chain(emit):
                    nonlocal nops
                    if nops:
                        v.wait_ge(vop_sem, nops)
                    emit().then_inc(vop_sem, 1)
                    nops += 1

                U = gbuf[:, 0, entries[0][1]]
                if m == 1:
                    W = gbuf[:, 1, entries[1][1]]
                    chain(lambda: v.tensor_mul(tsum[:, :, :], U, W))
                else:
                    W1 = gbuf[:, 1, entries[1][1]]
                    W2 = gbuf[:, 2, entries[2][1]]
                    chain(lambda: v.tensor_add(tsum[:, :, :], W1, W2))
                    for jj in range(3, m + 1):
                        Wj = gbuf[:, jj, entries[jj][1]]
                        chain(
                            lambda Wj=Wj: v.tensor_add(
                                tsum[:, :, :], tsum[:, :, :], Wj
                            )
                        )
                    chain(lambda: v.tensor_mul(tsum[:, :, :], tsum[:, :, :], U))
                if bi == 0:
                    chain(lambda: v.tensor_copy(acc[:, :, :], tsum[:, :, :]))
                else:
                    chain(
                        lambda: v.tensor_add(acc[:, :, :], acc[:, :, :], tsum[:, :, :])
                    )

        @block.sync
        def _(sy: bass.BassEngine):
            sy.wait_ge(vop_sem, total_ops)
            sy.dma_start(out[:, :, :], acc[:, :, :]).then_inc(o_sem, 16)
            sy.wait_ge(o_sem, 16)

    nc.compile()
    _NC_CACHE[key] = nc
    return nc


def kernel(edge_index: np.ndarray, node_ids: np.ndarray) -> np.ndarray:
    edge_index = np.asarray(edge_index)
    node_ids = np.asarray(node_ids, dtype=np.float32)
    N, D = node_ids.shape
    assert (N, D) == (N_NODES, HV_DIM)

    a, b = _host_indices(edge_index)
    schedule, idx_cols = _plan_groups(a, b)

    in_maps = []
    for c in range(N_CORES):
        tbl = np.zeros((NROWS, EW), dtype=np.float32)
        lo = c * EW
        hi = min(lo + EW, D)
        tbl[:N_NODES, : hi - lo] = node_ids[:, lo:hi]
        in_maps.append({"table": tbl, "idx": idx_cols})

    nc = _build_nc(schedule, idx_cols.shape[1])

    from concourse.bass_utils import run_bass_kernel_spmd

    res = run_bass_kernel_spmd(nc, in_maps, core_ids=list(range(N_CORES)))

    out = np.empty(D, dtype=np.float32)
    for c in range(N_CORES):
        accum = res.results[c]["out"].astype(np.float64).sum(axis=(0, 1))
        lo = c * EW
        hi = min(lo + EW, D)
        out[lo:hi] = accum[: hi - lo].astype(np.float32)
    return out


# revision 7
# speedup vs baseline: 6.0337x; 6.0337x over previous
"""Candidate kernel.py: grouped-gather version (self-contained)."""

import math

import numpy as np

N_NODES = 10000
HV_DIM = 10000
ALPHA = 0.85
PR_ITERS = 10

NROWS = N_NODES + 1  # row N_NODES is all-zeros (padding target)
EW = 1280  # per-core D-slice width (8 * 1280 = 10240 >= 10000)
BLK = 256  # groups per gather block
G = BLK // 128
N_CORES = 8
KINDS = (4, 2, 1)
MAXM = max(KINDS)


def _host_indices(edge_index: np.ndarray) -> tuple[np.ndarray, np.ndarray]:
    """Pagerank -> rank permutation -> deduped undirected edge endpoint rows.
    Bit-exact replica of the reference ops on the CPU jax backend."""
    import jax
    import jax.numpy as jnp
    from jax import lax

    N = N_NODES
    cpu = jax.devices("cpu")[0]

    def _impl(edge_index):
        row, col = edge_index[0], edge_index[1]
        dtype = jnp.float32
        counts = jax.ops.segment_sum(
            jnp.ones_like(col, dtype=dtype), col, num_segments=N
        )
        vals = ALPHA / counts[col]
        p = jnp.asarray((1.0 - ALPHA) / N, dtype=dtype)
        v0 = jnp.full((N,), 1.0 / N, dtype=dtype)

        def step(v, _):
            v = jax.ops.segment_sum(vals * v[col], row, num_segments=N) + p
            return v, None

        v, _ = lax.scan(step, v0, None, length=PR_ITERS)

        perm = jnp.argsort(v)
        inv = (
            jnp.zeros((N,), dtype=jnp.int32)
            .at[perm]
            .set(jnp.arange(N, dtype=jnp.int32))
        )

        lo = jnp.minimum(row, col)
        hi = jnp.maximum(row, col)
        ekey = lo * jnp.int32(N) + hi
        order = jnp.argsort(ekey)
        skey = ekey[order]
        first = jnp.concatenate([jnp.ones((1,), dtype=bool), skey[1:] != skey[:-1]])
        slo = lo[order]
        shi = hi[order]
        return inv[slo], inv[shi], first

    with jax.default_device(cpu):
        ei = jax.device_put(np.asarray(edge_index), cpu)
        try:
            fn = jax.jit(_impl, backend="cpu")
        except TypeError:
            fn = jax.jit(_impl)
        ia, ib, first = fn(ei)
        ia, ib, first = np.asarray(ia), np.asarray(ib), np.asarray(first)
    return ia[first], ib[first]


def _plan_groups(a: np.ndarray, b: np.ndarray):
    """Orient each edge toward its higher-degree endpoint, chunk each node's
    partner list into quads/pairs/singles (then regroup leftover singles by
    the other endpoint), and emit the flat gather schedule.

    Returns (schedule, idx_cols): schedule = [(m, [col offsets of U, W1..Wm])],
    idx_cols = packed [128, S] int16 gather-index tensor."""
    deg = np.bincount(np.concatenate([a, b]), minlength=NROWS)
    swap = deg[b] > deg[a]
    u = np.where(swap, b, a)
    w = np.where(swap, a, b)
    order = np.argsort(u, kind="stable")
    u, w = u[order], w[order]

    def chunk_runs(keys, vals):
        out = {m: [] for m in KINDS}
        i = 0
        n = len(keys)
        while i < n:
            j = i
            while j < n and keys[j] == keys[i]:
                j += 1
            run = vals[i:j]
            k = 0
            for m in KINDS:
                while len(run) - k >= m:
                    out[m].append((keys[i], run[k : k + m]))
                    k += m
            i = j
        return out

    groups = chunk_runs(u, w)
    singles = groups[1]
    if singles:  # products commute: regroup singles by the partner endpoint
        su = np.array([g[0] for g in singles])
        sw = np.array([g[1][0] for g in singles])
        o2 = np.argsort(sw, kind="stable")
        regrouped = chunk_runs(sw[o2], su[o2])
        groups[1] = regrouped.pop(1)
        for m in KINDS[:-1]:
            groups[m].extend(regrouped[m])

    idx_stream = []
    schedule = []
    col = 0

    def emit_gather(vals):
        nonlocal col
        arr = np.full(BLK, N_NODES, dtype=np.int16)
        arr[: len(vals)] = vals
        idx_stream.append(arr)
        off = col
        col += BLK // 16
        return off

    for m in KINDS:
        g = groups[m]
        for bi in range(math.ceil(len(g) / BLK)):
            chunk = g[bi * BLK : (bi + 1) * BLK]
            offs = [emit_gather(np.array([c[0] for c in chunk], dtype=np.int16))]
            for j in range(m):
                offs.append(
                    emit_gather(np.array([c[1][j] for c in chunk], dtype=np.int16))
                )
            schedule.append((m, offs))

    idx_cols = np.zeros((128, col), dtype=np.int16)
    for k, arr in enumerate(idx_stream):
        wrapped = arr.reshape(-1, 16).T
        idx_cols[:, k * 16 : (k + 1) * 16] = np.tile(wrapped, (8, 1))
    return schedule, idx_cols


_NC_CACHE: dict = {}


DEPTH = (6, 4, 4, 2, 2)  # per-stream slot depth: U is hottest (every block)


def _build_nc(schedule, scols: int, bufs=DEPTH):
    """Per-core bass program processing the gather schedule.
    Per block (kind m): gather U + W1..Wm (BLK idxs each), then DVE computes
    acc += U * (W1 + ... + Wm)."""
    depth = list(bufs) if isinstance(bufs, (tuple, list)) else [bufs] * (MAXM + 1)
    key = (scols, tuple(depth), tuple((m, tuple(o)) for m, o in schedule))
    if key in _NC_CACHE:
        return _NC_CACHE[key]

    from contextlib import ExitStack

    import concourse.bacc as bacc
    import concourse.bass as bass
    import concourse.mybir as mybir
    from concourse import library_config
    from concourse._compat import get_trn_type

    DT = mybir.dt.float32
    nc = bacc.Bacc(get_trn_type() or "TRN2")
    table = nc.dram_tensor("table", [NROWS, EW], DT, kind="ExternalInput")
    idx = nc.dram_tensor("idx", [128, scols], mybir.dt.int16, kind="ExternalInput")
    out = nc.dram_tensor("out", [128, G, EW], DT, kind="ExternalOutput")

    ops_of = lambda m: 2 if m == 1 else m + 1  # noqa: E731
    done_ops = []
    tot = 0
    for m, _ in schedule:
        tot += ops_of(m)
        done_ops.append(tot)
    total_ops = tot
    NSTREAM = MAXM + 1
    assert len(depth) == NSTREAM

    with (
        ExitStack() as stack,
        nc.sbuf_tensor("idx_sb", [128, scols], mybir.dt.int16) as idx_sb,
        nc.sbuf_tensor("gbuf", [128, sum(depth), G, EW], DT) as gbuf,
        nc.sbuf_tensor("tsum", [128, G, EW], DT) as tsum,
        nc.sbuf_tensor("acc", [128, G, EW], DT) as acc,
        nc.semaphore("io_sem") as io_sem,
        nc.semaphore("o_sem") as o_sem,
        nc.semaphore("vop_sem") as vop_sem,
        nc.Block() as block,
    ):
        base = [sum(depth[:j]) for j in range(NSTREAM)]
        gsem = [
            [stack.enter_context(nc.semaphore(f"g{j}_{s}")) for s in range(depth[j])]  # noqa: ANT232
            for j in range(NSTREAM)
        ]

        # deterministic (stream, slot) rotation shared by both engine streams
        def walk():
            use = [[0] * depth[jx] for jx in range(NSTREAM)]
            hist = [[[] for _ in range(depth[jx])] for jx in range(NSTREAM)]
            for bi, (m, offs) in enumerate(schedule):
                entries = []
                for jj in range(m + 1):
                    cnt = sum(use[jj])
                    s = cnt % depth[jj]
                    use[jj][s] += 1
                    prior = hist[jj][s][:]
                    hist[jj][s].append(bi)
                    entries.append((jj, s, use[jj][s], prior, offs[jj]))
                yield bi, m, entries

        @block.gpsimd
        def _(gp: bass.BassGpSimd):
            gp.load_library(library_config.mlp)
            gp.dma_start(idx_sb[:, :], idx[:, :]).then_inc(io_sem, 16)
            gp.wait_ge(io_sem, 16)
            for bi, m, entries in walk():
                for jj, s, k, prior, off in entries:
                    if prior:
                        # slot reuse: DVE must have consumed block prior[-1]
                        gp.wait_ge(vop_sem, done_ops[prior[-1]])
                    gp.dma_gather(
                        gbuf[:, base[jj] + s],
                        table[:, :],
                        idx_sb[:, bass.ds(off, BLK // 16)],
                        BLK,
                        BLK,
                        EW,
                    ).then_inc(gsem[jj][s], 16)

        @block.vector
        def _(v: bass.BassEngine):
            nops = 0
            for bi, m, entries in walk():
                for jj, s, k, prior, off in entries:
                    v.wait_ge(gsem[jj][s], 16 * k)

                def chain(emit):
                    nonlocal nops
                    if nops:
                        v.wait_ge(vop_sem, nops)
                    emit().then_inc(vop_sem, 1)
                    nops += 1

                U = gbuf[:, base[0] + entries[0][1]]
                if m == 1:
                    W = gbuf[:, base[1] + entries[1][1]]
                    chain(lambda: v.tensor_mul(tsum[:, :, :], U, W))
                else:
                    W1 = gbuf[:, base[1] + entries[1][1]]
                    W2 = gbuf[:, base[2] + entries[2][1]]
                    chain(lambda: v.tensor_add(tsum[:, :, :], W1, W2))
                    for jj in range(3, m + 1):
                        Wj = gbuf[:, base[jj] + entries[jj][1]]
                        chain(
                            lambda Wj=Wj: v.tensor_add(
                                tsum[:, :, :], tsum[:, :, :], Wj
                            )
                        )
                    chain(lambda: v.tensor_mul(tsum[:, :, :], tsum[:, :, :], U))
                if bi == 0:
                    chain(lambda: v.tensor_copy(acc[:, :, :], tsum[:, :, :]))
                else:
                    chain(
                        lambda: v.tensor_add(acc[:, :, :], acc[:, :, :], tsum[:, :, :])
                    )

        @block.sync
        def _(sy: bass.BassEngine):
            sy.wait_ge(vop_sem, total_ops)
            sy.dma_start(out[:, :, :], acc[:, :, :]).then_inc(o_sem, 16)
            sy.wait_ge(o_sem, 16)

    nc.compile()
    _NC_CACHE[key] = nc
    return nc


def kernel(edge_index: np.ndarray, node_ids: np.ndarray) -> np.ndarray:
    edge_index = np.asarray(edge_index)
    node_ids = np.asarray(node_ids, dtype=np.float32)
    N, D = node_ids.shape
    assert (N, D) == (N_NODES, HV_DIM)

    a, b = _host_indices(edge_index)
    schedule, idx_cols = _plan_groups(a, b)

    in_maps = []
    for c in range(N_CORES):
        tbl = np.zeros((NROWS, EW), dtype=np.float32)
        lo = c * EW
        hi = min(lo + EW, D)
        tbl[:N_NODES, : hi - lo] = node_ids[:, lo:hi]
        in_maps.append({"table": tbl, "idx": idx_cols})

    nc = _build_nc(schedule, idx_cols.shape[1])

    from concourse.bass_utils import run_bass_kernel_spmd

    res = run_bass_kernel_spmd(nc, in_maps, core_ids=list(range(N_CORES)))

    out = np.empty(D, dtype=np.float32)
    for c in range(N_CORES):
        accum = res.results[c]["out"].astype(np.float64).sum(axis=(0, 1))
        lo = c * EW
        hi = min(lo + EW, D)
        out[lo:hi] = accum[: hi - lo].astype(np.float32)
    return out
